# revision 17
# baseline (speedup 1.0000x reference)
"""Multi-head attention (B=2, S=2048, d_model=1024, 16 heads, dk=dv=64) on
8 Trainium2 NeuronCores.

Sharding: core = (batch, group-of-4-heads).  Each core projects q/k/v for its
4 heads (full sequence of its batch), runs softmax(q k^T) v without masking
(the harness mask is always all-True), applies its 256 rows of Wo, and returns
a partial [S, d_model] output.  The host sums the 4 partials per batch
(row-parallel Wo => host-side reduction instead of a device all-reduce).

v3 schedule:
  * Inputs stream in 512-column chunks (k first), projections chase the DMA.
  * q-projection for query-quarter c runs at the top of attention block c so
    it overlaps earlier blocks' ScalarE-bound softmax work.
  * Attention: 8 blocks of (head-pair, 512-query quarter).  Scores for the
    two heads of a pair target PE row groups 0/64 and run concurrently.
  * exp: ScalarE ACTIVATE for most key-tiles; a fixed subset is offloaded to
    VectorE as a Schraudolph bit-trick exp (affine fp32->int32 cast, then a
    bitcast copy to bf16), balancing the two engines.  The softmax
    normalization cancels most of the ~2% Schraudolph error.
  * Per-head-pair denominator batch: SBUF->SBUF DMA stages the two ones-row
    sums onto partition 0; one reciprocal_approx_fast inverts both.
  * Output projection + output DMA interleave per query quarter.
  * PSUM: proj/oproj share 2 banks, score tiles 2x2, av 2x1 -> exactly 8.
"""

import numpy as np

import concourse.bass as bass
import concourse.mybir as mybir
import concourse.tile as tile
from concourse import bacc
from concourse.bass_utils import run_bass_kernel_spmd

P = 128
S = 2048
D = 1024
KT = D // P          # 8 k-tiles over d_model
NH = 4               # heads per core
DK = 64
NCORES = 8
CH = 512             # column chunk (keys/queries) for DMA + proj + attention
NCH = S // CH        # 4 chunks
F32 = mybir.dt.float32
I32 = mybir.dt.int32
BF16 = mybir.dt.bfloat16
AF = mybir.ActivationFunctionType

# Schraudolph exp: exp(x) ~= bitcast_f32(int32(A*x + B)); applied on these
# key-tile indices (DVE), the rest stay on ScalarE's exact exp.
SCH_A = float(2 ** 23 / np.log(2.0))
SCH_B = float(127 * 2 ** 23 - 366000)
SCH_J = ()

_CACHE: dict = {}
LAST_RESULTS = None  # test harness peeks at this for exec_time_ns


def _build_nc():
    nc = bacc.Bacc("TRN2", target_bir_lowering=False, num_devices=NCORES)

    qT = nc.dram_tensor("qT", [D, S], BF16, kind="ExternalInput").ap()
    kT = nc.dram_tensor("kT", [D, S], BF16, kind="ExternalInput").ap()
    vT = nc.dram_tensor("vT", [D, S], BF16, kind="ExternalInput").ap()
    wq = nc.dram_tensor("wq", [D, NH * DK], BF16, kind="ExternalInput").ap()
    wk = nc.dram_tensor("wk", [D, NH * DK], BF16, kind="ExternalInput").ap()
    wv = nc.dram_tensor("wv", [D, NH * DK], BF16, kind="ExternalInput").ap()
    wo = nc.dram_tensor("wo", [NH * DK, D], BF16, kind="ExternalInput").ap()
    out = nc.dram_tensor("outT", [D, S], F32, kind="ExternalOutput").ap()

    with tile.TileContext(nc) as tc:
        _build_body(nc, tc, qT, kT, vT, wq, wk, wv, wo, out)
    nc.compile()
    return nc


def _build_body(nc, tc, qT, kT, vT, wq, wk, wv, wo, out):
    from contextlib import ExitStack

    with ExitStack() as ctx:
        constp = ctx.enter_context(tc.tile_pool(name="const", bufs=1))

        # ---- persistent SBUF tensors -----------------------------------
        wq_s = constp.tile([P, KT, NH * DK], BF16)
        wk_s = constp.tile([P, KT, NH * DK], BF16)
        wv_s = constp.tile([P, KT, NH * DK], BF16)
        wo_s = constp.tile([P, 2, D], BF16)

        qhT = constp.tile([P, 2, S], BF16)   # [2 heads stacked, pair, S]
        khT = constp.tile([P, 2, S], BF16)
        # vh + ones column per head: [s_tile_part, s_tile, head, dv+1]
        vh_s = constp.tile([P, 16, NH, DK + 1], BF16)
        nc.any.memset(vh_s[:, :, :, DK], 1.0)
        # pair-stacked scaled attention output, transposed: [2*dv, S]
        avT = [constp.tile([P, S], BF16, name=f"avT{pr}") for pr in range(2)]
        # softmax denominators, all staged to partition 0: [1, head, ihq, CH]
        den_all = constp.tile([1, NH, NCH, CH], F32)
        rec_all = constp.tile([1, NH, NCH, CH], F32)

        # ---- input DMA: first chunk of k/v/q first, q tails last -------
        xstream = ctx.enter_context(tc.tile_pool(name="xstream", bufs=6))
        xch: dict = {}

        def load_chunk(eng, name, src_, c):
            cs = slice(c * CH, (c + 1) * CH)
            t = xstream.tile([P, KT, CH], BF16, tag="xs", name=f"x{name}{c}")
            eng.dma_start(t, src_[:, cs].rearrange("(kt p) s -> p kt s", p=P))
            xch[name, c] = t

        nc.sync.dma_start(wk_s, wk.rearrange("(kt p) n -> p kt n", p=P))
        nc.gpsimd.dma_start(wv_s, wv.rearrange("(kt p) n -> p kt n", p=P))
        load_chunk(nc.sync, "k", kT, 0)
        load_chunk(nc.gpsimd, "v", vT, 0)
        nc.gpsimd.dma_start(wq_s, wq.rearrange("(kt p) n -> p kt n", p=P))
        load_chunk(nc.gpsimd, "q", qT, 0)
        nc.gpsimd.dma_start(wo_s, wo.rearrange("(pair p) n -> p pair n", p=P))
        for c in range(1, NCH):
            load_chunk(nc.sync, "k", kT, c)
            load_chunk(nc.gpsimd, "v", vT, c)
        for c in range(1, NCH):
            load_chunk(nc.sync, "q", qT, c)

        # pproj: shared 1-bank slots for q/k/v projections AND the output
        # projection (lifetimes interleave but tags share the 2 slots).
        pproj = ctx.enter_context(
            tc.tile_pool(name="pproj", bufs=2, space="PSUM"))
        pst = ctx.enter_context(tc.tile_pool(name="pst", bufs=2, space="PSUM"))
        pav = ctx.enter_context(tc.tile_pool(name="pav", bufs=1, space="PSUM"))
        attsb = ctx.enter_context(tc.tile_pool(name="attsb", bufs=5))
        nrmsb = ctx.enter_context(tc.tile_pool(name="nrmsb", bufs=2))
        schsb = ctx.enter_context(tc.tile_pool(name="schsb", bufs=2))
        osb = ctx.enter_context(tc.tile_pool(name="osb", bufs=2))

        def qk_proj(name, wsb, dst, c):
            xc = xch[name, c]
            cs = slice(c * CH, (c + 1) * CH)
            for pr in range(2):
                ps = pproj.tile([P, CH], F32, tag="pj", name="pj")
                for kt in range(KT):
                    nc.tensor.matmul(
                        ps,
                        wsb[:, kt, pr * P:(pr + 1) * P],
                        xc[:, kt, :],
                        start=(kt == 0),
                        stop=(kt == KT - 1),
                    )
                # all PSUM->SBUF copies stay off ScalarE: it runs exp only
                nc.vector.tensor_copy(dst[:, pr, cs], ps)

        def v_proj(c):
            # stationary = v chunk [128d, 128keys]; moving = Wv [128d, 256]
            xc = xch["v", c]
            for sst in range(CH // P):
                st = c * (CH // P) + sst
                vp = pproj.tile([P, NH * DK], F32, tag="pj", name="pj")
                for kt in range(KT):
                    nc.tensor.matmul(
                        vp,
                        xc[:, kt, sst * P:(sst + 1) * P],
                        wv_s[:, kt, :],
                        start=(kt == 0),
                        stop=(kt == KT - 1),
                    )
                src = vp.rearrange("p (h d) -> p h d", h=NH)
                nc.vector.tensor_copy(vh_s[:, st, :, 0:DK], src)

        # ---- attention block plumbing ----------------------------------
        LAG = 3

        def attn_start(pr, ihq):
            av_A = pav.tile([DK + 1, CH], F32, tag="avA", name="av_A")
            av_B = pav.tile([DK + 1, CH], F32, tag="avB", name="av_B")
            return {"pr": pr, "ihq": ihq, "av_A": av_A, "av_B": av_B,
                    "avq": []}

        def _av_mm(st, j, pt):
            pr = st["pr"]
            nc.tensor.matmul(
                st["av_A"],
                vh_s[:, j, 2 * pr, :], pt[:, 0:CH],
                start=(j == 0), stop=(j == 15),
            )
            nc.tensor.matmul(
                st["av_B"],
                vh_s[:, j, 2 * pr + 1, :], pt[:, CH:2 * CH],
                start=(j == 0), stop=(j == 15),
            )

        def attn_js(st, j0, j1):
            pr, ihq = st["pr"], st["ihq"]
            qs = slice(ihq * CH, (ihq + 1) * CH)
            for j in range(j0, j1):
                js = slice(j * P, (j + 1) * P)
                stq = pst.tile([P, 2 * CH], F32, tag="st", name="stq")
                # two heads -> different PE row groups, run concurrently
                nc.tensor.matmul(
                    stq[:, 0:CH],
                    khT[0:DK, pr, js], qhT[0:DK, pr, qs],
                    start=True, stop=True,
                )
                nc.tensor.matmul(
                    stq[:, CH:2 * CH],
                    khT[DK:P, pr, js], qhT[DK:P, pr, qs],
                    start=True, stop=True,
                )
                pt = attsb.tile([P, 2 * CH], BF16, tag="pt", name="pt")
                if j in SCH_J:
                    it = schsb.tile([P, 2 * CH], I32, tag="sch", name="it")
                    nc.vector.tensor_scalar(
                        out=it, in0=stq, scalar1=SCH_A, scalar2=SCH_B,
                        op0=mybir.AluOpType.mult, op1=mybir.AluOpType.add,
                    )
                    nc.vector.tensor_copy(pt, it.bitcast(F32))
                else:
                    nc.scalar.activation(pt, stq, AF.Exp)
                # av matmuls accumulate (commutative over j) and are emitted
                # LAG tiles late so a slow exp never head-of-line blocks the
                # PE FIFO (which would starve whichever engine exps next).
                st["avq"].append((j, pt))
                if len(st["avq"]) > LAG:
                    _av_mm(st, *st["avq"].pop(0))

        def attn_finish(st):
            pr, ihq = st["pr"], st["ihq"]
            qs = slice(ihq * CH, (ihq + 1) * CH)
            for item in st["avq"]:
                _av_mm(st, *item)
            for h, av in enumerate((st["av_A"], st["av_B"])):
                r = 2 * pr + h
                av_sb = nrmsb.tile([DK + 1, CH], F32, tag=f"avsb{r}",
                                   name="av_sb")
                nc.vector.tensor_copy(av_sb, av)
                # stage the denominator row (partition 64 -> partition 0)
                nc.sync.dma_start(den_all[0:1, r, ihq, :],
                                  av_sb[DK:DK + 1, :])
                xch[("avsb", pr, h)] = av_sb
            # per-pair reciprocal so pr0's normalization overlaps pr1's
            # attention instead of waiting for the whole quarter
            nc.vector.reciprocal_approx_fast(
                rec_all[0:1, 2 * pr:2 * pr + 2, ihq, :],
                den_all[0:1, 2 * pr:2 * pr + 2, ihq, :])
            for h in range(2):
                r = 2 * pr + h
                av_sb = xch[("avsb", pr, h)]
                bcs = nrmsb.tile([DK, CH], F32, tag="bcs", name="bcs")
                nc.gpsimd.partition_broadcast(bcs, rec_all[0:1, r, ihq, :])
                nc.vector.tensor_mul(
                    out=avT[pr][h * DK:(h + 1) * DK, qs],
                    in0=av_sb[0:DK, :],
                    in1=bcs,
                )

        def oproj(ihq, tail=False):
            qs = slice(ihq * CH, (ihq + 1) * CH)
            for dc in range(8):
                ds_ = slice(dc * P, (dc + 1) * P)
                ops = pproj.tile([P, CH], F32, tag="pj", name="pj")
                for pair in range(2):
                    nc.tensor.matmul(
                        ops,
                        wo_s[:, pair, ds_],
                        avT[pair][:, qs],
                        start=(pair == 0), stop=(pair == 1),
                    )
                oto = osb.tile([P, CH], F32, tag="oto", name="oto")
                if tail and dc % 2 == 0:
                    nc.scalar.copy(oto, ops)
                else:
                    nc.vector.tensor_copy(oto, ops)
                nc.sync.dma_start(out[ds_, qs], oto)

        # ---- fused schedule --------------------------------------------
        # quarter 0 / pair 0 chases the k/v projection chunk by chunk: its
        # first score tiles only need keys 0:512, so ScalarE starts exping
        # ~30us before the projections finish.
        kv_done = [False] * NCH

        def kv_chunk(c):
            if not kv_done[c]:
                qk_proj("k", wk_s, khT, c)
                v_proj(c)
                kv_done[c] = True

        kv_chunk(0)
        qk_proj("q", wq_s, qhT, 0)
        st00 = attn_start(0, 0)
        for c in range(1, NCH):
            attn_js(st00, (c - 1) * 4, c * 4)
            kv_chunk(c)
        attn_js(st00, 12, 16)
        attn_finish(st00)

        for ihq in range(NCH):          # query quarter (512 queries)
            if ihq > 0:
                qk_proj("q", wq_s, qhT, ihq)
                st = attn_start(0, ihq)
                attn_js(st, 0, 16)
                attn_finish(st)
            if ihq > 0:
                oproj(ihq - 1)
            st = attn_start(1, ihq)
            attn_js(st, 0, 16)
            attn_finish(st)
        oproj(NCH - 1, tail=True)


def kernel(q, k, v, mask, Wq, Wk, Wv, Wo, _trace=False, _tmpdir=None):
    """Full inputs in, full output out. mask is all-True by construction of
    the problem's input spec and is ignored (dense softmax)."""
    global LAST_RESULTS

    import ml_dtypes

    bf16 = ml_dtypes.bfloat16
    q = np.asarray(q, dtype=np.float32)
    k = np.asarray(k, dtype=np.float32)
    v = np.asarray(v, dtype=np.float32)
    Wq = np.asarray(Wq, dtype=bf16)
    Wk = np.asarray(Wk, dtype=bf16)
    Wv = np.asarray(Wv, dtype=bf16)
    Wo = np.asarray(Wo, dtype=bf16)
    B = q.shape[0]

    if "nc" not in _CACHE:
        _CACHE["nc"] = _build_nc()
    nc = _CACHE["nc"]

    qTb = [np.ascontiguousarray(q[b].T).astype(bf16) for b in range(B)]
    kTb = [np.ascontiguousarray(k[b].T).astype(bf16) for b in range(B)]
    vTb = [np.ascontiguousarray(v[b].T).astype(bf16) for b in range(B)]

    in_maps = []
    for core in range(NCORES):
        b, hg = core // 4, core % 4
        cs = slice(hg * NH * DK, (hg + 1) * NH * DK)
        in_maps.append({
            "qT": qTb[b],
            "kT": kTb[b],
            "vT": vTb[b],
            "wq": np.ascontiguousarray(Wq[:, cs]),
            "wk": np.ascontiguousarray(Wk[:, cs]),
            "wv": np.ascontiguousarray(Wv[:, cs]),
            "wo": np.ascontiguousarray(Wo[cs, :]),
        })

    res = run_bass_kernel_spmd(
        nc, in_maps, core_ids=list(range(NCORES)),
        trace=_trace, tmpdir=_tmpdir,
    )
    LAST_RESULTS = res

    fullT = np.zeros((B, D, S), dtype=np.float32)
    for core in range(NCORES):
        fullT[core // 4] += res.results[core]["outT"]
    return np.ascontiguousarray(fullT.transpose(0, 2, 1))


# revision 19
# speedup vs baseline: 1.0063x; 1.0063x over previous
"""Multi-head attention (B=2, S=2048, d_model=1024, 16 heads, dk=dv=64) on
8 Trainium2 NeuronCores.

Sharding: core = (batch, group-of-4-heads).  Each core projects q/k/v for its
4 heads (full sequence of its batch), runs softmax(q k^T) v without masking
(the harness mask is always all-True), applies its 256 rows of Wo, and returns
a partial [S, d_model] output.  The host sums the 4 partials per batch
(row-parallel Wo => host-side reduction instead of a device all-reduce).

v3 schedule:
  * Inputs stream in 512-column chunks (k first), projections chase the DMA.
  * q-projection for query-quarter c runs at the top of attention block c so
    it overlaps earlier blocks' ScalarE-bound softmax work.
  * Attention: 8 blocks of (head-pair, 512-query quarter).  Scores for the
    two heads of a pair target PE row groups 0/64 and run concurrently.
  * exp: ScalarE ACTIVATE for most key-tiles; a fixed subset is offloaded to
    VectorE as a Schraudolph bit-trick exp (affine fp32->int32 cast, then a
    bitcast copy to bf16), balancing the two engines.  The softmax
    normalization cancels most of the ~2% Schraudolph error.
  * Per-head-pair denominator batch: SBUF->SBUF DMA stages the two ones-row
    sums onto partition 0; one reciprocal_approx_fast inverts both.
  * Output projection + output DMA interleave per query quarter.
  * PSUM: proj/oproj share 2 banks, score tiles 2x2, av 2x1 -> exactly 8.
"""

import numpy as np

import concourse.bass as bass
import concourse.mybir as mybir
import concourse.tile as tile
from concourse import bacc
from concourse.bass_utils import run_bass_kernel_spmd

P = 128
S = 2048
D = 1024
KT = D // P          # 8 k-tiles over d_model
NH = 4               # heads per core
DK = 64
NCORES = 8
CH = 512             # column chunk (keys/queries) for DMA + proj + attention
NCH = S // CH        # 4 chunks
F32 = mybir.dt.float32
I32 = mybir.dt.int32
BF16 = mybir.dt.bfloat16
AF = mybir.ActivationFunctionType

# Schraudolph exp: exp(x) ~= bitcast_f32(int32(A*x + B)); applied on these
# key-tile indices (DVE), the rest stay on ScalarE's exact exp.
SCH_A = float(2 ** 23 / np.log(2.0))
SCH_B = float(127 * 2 ** 23 - 366000)
SCH_J = ()

_CACHE: dict = {}
LAST_RESULTS = None  # test harness peeks at this for exec_time_ns


def _build_nc():
    nc = bacc.Bacc("TRN2", target_bir_lowering=False, num_devices=NCORES)

    qT = nc.dram_tensor("qT", [D, S], BF16, kind="ExternalInput").ap()
    kT = nc.dram_tensor("kT", [D, S], BF16, kind="ExternalInput").ap()
    vT = nc.dram_tensor("vT", [D, S], BF16, kind="ExternalInput").ap()
    wq = nc.dram_tensor("wq", [D, NH * DK], BF16, kind="ExternalInput").ap()
    wk = nc.dram_tensor("wk", [D, NH * DK], BF16, kind="ExternalInput").ap()
    wv = nc.dram_tensor("wv", [D, NH * DK], BF16, kind="ExternalInput").ap()
    wo = nc.dram_tensor("wo", [NH * DK, D], BF16, kind="ExternalInput").ap()
    out = nc.dram_tensor("outT", [D, S], F32, kind="ExternalOutput").ap()

    with tile.TileContext(nc) as tc:
        _build_body(nc, tc, qT, kT, vT, wq, wk, wv, wo, out)
    nc.compile()
    return nc


def _build_body(nc, tc, qT, kT, vT, wq, wk, wv, wo, out):
    from contextlib import ExitStack

    with ExitStack() as ctx:
        constp = ctx.enter_context(tc.tile_pool(name="const", bufs=1))

        # ---- persistent SBUF tensors -----------------------------------
        wq_s = constp.tile([P, KT, NH * DK], BF16)
        wk_s = constp.tile([P, KT, NH * DK], BF16)
        wv_s = constp.tile([P, KT, NH * DK], BF16)
        wo_s = constp.tile([P, 2, D], BF16)

        qhT = constp.tile([P, 2, S], BF16)   # [2 heads stacked, pair, S]
        khT = constp.tile([P, 2, S], BF16)
        # vh + ones column per head: [s_tile_part, s_tile, head, dv+1]
        vh_s = constp.tile([P, 16, NH, DK + 1], BF16)
        nc.any.memset(vh_s[:, :, :, DK], 1.0)
        # pair-stacked scaled attention output, transposed: [2*dv, S]
        avT = [constp.tile([P, S], BF16, name=f"avT{pr}") for pr in range(2)]
        # softmax denominators, all staged to partition 0: [1, head, ihq, CH]
        den_all = constp.tile([1, NH, NCH, CH], F32)
        rec_all = constp.tile([1, NH, NCH, CH], F32)

        # ---- input DMA: first chunk of k/v/q first, q tails last -------
        xstream = ctx.enter_context(tc.tile_pool(name="xstream", bufs=6))
        xch: dict = {}

        def load_chunk(name, src_, c):
            cs = slice(c * CH, (c + 1) * CH)
            t = xstream.tile([P, KT, CH], BF16, tag="xs", name=f"x{name}{c}")
            nc.sync.dma_start(t, src_[:, cs].rearrange("(kt p) s -> p kt s", p=P))
            xch[name, c] = t

        nc.sync.dma_start(wk_s, wk.rearrange("(kt p) n -> p kt n", p=P))
        load_chunk("k", kT, 0)
        nc.sync.dma_start(wv_s, wv.rearrange("(kt p) n -> p kt n", p=P))
        load_chunk("v", vT, 0)
        nc.sync.dma_start(wq_s, wq.rearrange("(kt p) n -> p kt n", p=P))
        load_chunk("q", qT, 0)
        nc.sync.dma_start(wo_s, wo.rearrange("(pair p) n -> p pair n", p=P))
        for c in range(1, NCH):
            load_chunk("k", kT, c)
            load_chunk("v", vT, c)
        for c in range(1, NCH):
            load_chunk("q", qT, c)

        # pproj: shared 1-bank slots for q/k/v projections AND the output
        # projection (lifetimes interleave but tags share the 2 slots).
        pproj = ctx.enter_context(
            tc.tile_pool(name="pproj", bufs=2, space="PSUM"))
        pst = ctx.enter_context(tc.tile_pool(name="pst", bufs=2, space="PSUM"))
        pav = ctx.enter_context(tc.tile_pool(name="pav", bufs=1, space="PSUM"))
        attsb = ctx.enter_context(tc.tile_pool(name="attsb", bufs=5))
        nrmsb = ctx.enter_context(tc.tile_pool(name="nrmsb", bufs=2))
        schsb = ctx.enter_context(tc.tile_pool(name="schsb", bufs=2))
        osb = ctx.enter_context(tc.tile_pool(name="osb", bufs=2))

        def qk_proj(name, wsb, dst, c):
            xc = xch[name, c]
            cs = slice(c * CH, (c + 1) * CH)
            for pr in range(2):
                ps = pproj.tile([P, CH], F32, tag="pj", name="pj")
                for kt in range(KT):
                    nc.tensor.matmul(
                        ps,
                        wsb[:, kt, pr * P:(pr + 1) * P],
                        xc[:, kt, :],
                        start=(kt == 0),
                        stop=(kt == KT - 1),
                    )
                # all PSUM->SBUF copies stay off ScalarE: it runs exp only
                nc.vector.tensor_copy(dst[:, pr, cs], ps)

        def v_proj(c):
            # stationary = v chunk [128d, 128keys]; moving = Wv [128d, 256]
            xc = xch["v", c]
            for sst in range(CH // P):
                st = c * (CH // P) + sst
                vp = pproj.tile([P, NH * DK], F32, tag="pj", name="pj")
                for kt in range(KT):
                    nc.tensor.matmul(
                        vp,
                        xc[:, kt, sst * P:(sst + 1) * P],
                        wv_s[:, kt, :],
                        start=(kt == 0),
                        stop=(kt == KT - 1),
                    )
                src = vp.rearrange("p (h d) -> p h d", h=NH)
                nc.vector.tensor_copy(vh_s[:, st, :, 0:DK], src)

        # ---- attention block plumbing ----------------------------------
        LAG = 3

        def attn_start(pr, ihq):
            av_A = pav.tile([DK + 1, CH], F32, tag="avA", name="av_A")
            av_B = pav.tile([DK + 1, CH], F32, tag="avB", name="av_B")
            return {"pr": pr, "ihq": ihq, "av_A": av_A, "av_B": av_B,
                    "avq": []}

        def _av_mm(st, j, pt):
            pr = st["pr"]
            nc.tensor.matmul(
                st["av_A"],
                vh_s[:, j, 2 * pr, :], pt[:, 0:CH],
                start=(j == 0), stop=(j == 15),
            )
            nc.tensor.matmul(
                st["av_B"],
                vh_s[:, j, 2 * pr + 1, :], pt[:, CH:2 * CH],
                start=(j == 0), stop=(j == 15),
            )

        def attn_js(st, j0, j1):
            pr, ihq = st["pr"], st["ihq"]
            qs = slice(ihq * CH, (ihq + 1) * CH)
            for j in range(j0, j1):
                js = slice(j * P, (j + 1) * P)
                stq = pst.tile([P, 2 * CH], F32, tag="st", name="stq")
                # two heads -> different PE row groups, run concurrently
                nc.tensor.matmul(
                    stq[:, 0:CH],
                    khT[0:DK, pr, js], qhT[0:DK, pr, qs],
                    start=True, stop=True,
                )
                nc.tensor.matmul(
                    stq[:, CH:2 * CH],
                    khT[DK:P, pr, js], qhT[DK:P, pr, qs],
                    start=True, stop=True,
                )
                pt = attsb.tile([P, 2 * CH], BF16, tag="pt", name="pt")
                if j in SCH_J:
                    it = schsb.tile([P, 2 * CH], I32, tag="sch", name="it")
                    nc.vector.tensor_scalar(
                        out=it, in0=stq, scalar1=SCH_A, scalar2=SCH_B,
                        op0=mybir.AluOpType.mult, op1=mybir.AluOpType.add,
                    )
                    nc.vector.tensor_copy(pt, it.bitcast(F32))
                else:
                    nc.scalar.activation(pt, stq, AF.Exp)
                # av matmuls accumulate (commutative over j) and are emitted
                # LAG tiles late so a slow exp never head-of-line blocks the
                # PE FIFO (which would starve whichever engine exps next).
                st["avq"].append((j, pt))
                if len(st["avq"]) > LAG:
                    _av_mm(st, *st["avq"].pop(0))

        def _norm_head(pr, ihq, h, av_sb, qs):
            r = 2 * pr + h
            bcs = nrmsb.tile([DK, CH], F32, tag="bcs", name="bcs")
            nc.gpsimd.partition_broadcast(bcs, rec_all[0:1, r, ihq, :])
            nc.vector.tensor_mul(
                out=avT[pr][h * DK:(h + 1) * DK, qs],
                in0=av_sb[0:DK, :],
                in1=bcs,
            )

        def attn_finish(st, tail=False):
            pr, ihq = st["pr"], st["ihq"]
            qs = slice(ihq * CH, (ihq + 1) * CH)
            for item in st["avq"]:
                _av_mm(st, *item)
            for h, av in enumerate((st["av_A"], st["av_B"])):
                r = 2 * pr + h
                av_sb = nrmsb.tile([DK + 1, CH], F32, tag=f"avsb{r}",
                                   name="av_sb")
                nc.vector.tensor_copy(av_sb, av)
                # stage the denominator row (partition 64 -> partition 0)
                nc.sync.dma_start(den_all[0:1, r, ihq, :],
                                  av_sb[DK:DK + 1, :])
                xch[("avsb", pr, h)] = av_sb
                if tail:
                    # latency-critical: invert + scale each head as soon as
                    # its denominator lands instead of batching the pair
                    nc.vector.reciprocal_approx_fast(
                        rec_all[0:1, r, ihq, :], den_all[0:1, r, ihq, :])
                    _norm_head(pr, ihq, h, av_sb, qs)
            if tail:
                return
            # per-pair reciprocal so pr0's normalization overlaps pr1's
            # attention instead of waiting for the whole quarter
            nc.vector.reciprocal_approx_fast(
                rec_all[0:1, 2 * pr:2 * pr + 2, ihq, :],
                den_all[0:1, 2 * pr:2 * pr + 2, ihq, :])
            for h in range(2):
                _norm_head(pr, ihq, h, xch[("avsb", pr, h)], qs)

        def oproj(ihq, tail=False):
            qs = slice(ihq * CH, (ihq + 1) * CH)
            for dc in range(8):
                ds_ = slice(dc * P, (dc + 1) * P)
                ops = pproj.tile([P, CH], F32, tag="pj", name="pj")
                for pair in range(2):
                    nc.tensor.matmul(
                        ops,
                        wo_s[:, pair, ds_],
                        avT[pair][:, qs],
                        start=(pair == 0), stop=(pair == 1),
                    )
                oto = osb.tile([P, CH], F32, tag="oto", name="oto")
                if tail and dc % 2 == 0:
                    nc.scalar.copy(oto, ops)
                else:
                    nc.vector.tensor_copy(oto, ops)
                nc.sync.dma_start(out[ds_, qs], oto)

        # ---- fused schedule --------------------------------------------
        # quarter 0 / pair 0 chases the k/v projection chunk by chunk: its
        # first score tiles only need keys 0:512, so ScalarE starts exping
        # ~30us before the projections finish.
        kv_done = [False] * NCH

        def kv_chunk(c):
            if not kv_done[c]:
                qk_proj("k", wk_s, khT, c)
                v_proj(c)
                kv_done[c] = True

        qk_proj("k", wk_s, khT, 0)
        qk_proj("q", wq_s, qhT, 0)
        st00 = attn_start(0, 0)
        attn_js(st00, 0, 2)      # scores need only qh/kh; av lags by LAG
        v_proj(0)
        kv_done[0] = True
        for c in range(1, NCH):
            attn_js(st00, max(2, (c - 1) * 4), c * 4)
            kv_chunk(c)
        attn_js(st00, 12, 16)
        attn_finish(st00)

        for ihq in range(NCH):          # query quarter (512 queries)
            if ihq > 0:
                qk_proj("q", wq_s, qhT, ihq)
                st = attn_start(0, ihq)
                attn_js(st, 0, 16)
                attn_finish(st)
            if ihq > 0:
                oproj(ihq - 1)
            st = attn_start(1, ihq)
            attn_js(st, 0, 16)
            attn_finish(st, tail=(ihq == NCH - 1))
        oproj(NCH - 1, tail=True)


def kernel(q, k, v, mask, Wq, Wk, Wv, Wo, _trace=False, _tmpdir=None):
    """Full inputs in, full output out. mask is all-True by construction of
    the problem's input spec and is ignored (dense softmax)."""
    global LAST_RESULTS

    import ml_dtypes

    bf16 = ml_dtypes.bfloat16
    q = np.asarray(q, dtype=np.float32)
    k = np.asarray(k, dtype=np.float32)
    v = np.asarray(v, dtype=np.float32)
    Wq = np.asarray(Wq, dtype=bf16)
    Wk = np.asarray(Wk, dtype=bf16)
    Wv = np.asarray(Wv, dtype=bf16)
    Wo = np.asarray(Wo, dtype=bf16)
    B = q.shape[0]

    if "nc" not in _CACHE:
        _CACHE["nc"] = _build_nc()
    nc = _CACHE["nc"]

    qTb = [np.ascontiguousarray(q[b].T).astype(bf16) for b in range(B)]
    kTb = [np.ascontiguousarray(k[b].T).astype(bf16) for b in range(B)]
    vTb = [np.ascontiguousarray(v[b].T).astype(bf16) for b in range(B)]

    in_maps = []
    for core in range(NCORES):
        b, hg = core // 4, core % 4
        cs = slice(hg * NH * DK, (hg + 1) * NH * DK)
        in_maps.append({
            "qT": qTb[b],
            "kT": kTb[b],
            "vT": vTb[b],
            "wq": np.ascontiguousarray(Wq[:, cs]),
            "wk": np.ascontiguousarray(Wk[:, cs]),
            "wv": np.ascontiguousarray(Wv[:, cs]),
            "wo": np.ascontiguousarray(Wo[cs, :]),
        })

    res = run_bass_kernel_spmd(
        nc, in_maps, core_ids=list(range(NCORES)),
        trace=_trace, tmpdir=_tmpdir,
    )
    LAST_RESULTS = res

    fullT = np.zeros((B, D, S), dtype=np.float32)
    for core in range(NCORES):
        fullT[core // 4] += res.results[core]["outT"]
    return np.ascontiguousarray(fullT.transpose(0, 2, 1))


# revision 20
# speedup vs baseline: 1.0189x; 1.0125x over previous
"""Multi-head attention (B=2, S=2048, d_model=1024, 16 heads, dk=dv=64) on
8 Trainium2 NeuronCores.

Sharding: core = (batch, group-of-4-heads).  Each core projects q/k/v for its
4 heads (full sequence of its batch), runs softmax(q k^T) v without masking
(the harness mask is always all-True), applies its 256 rows of Wo, and returns
a partial [S, d_model] output.  The host sums the 4 partials per batch
(row-parallel Wo => host-side reduction instead of a device all-reduce).

v3 schedule:
  * Inputs stream in 512-column chunks (k first), projections chase the DMA.
  * q-projection for query-quarter c runs at the top of attention block c so
    it overlaps earlier blocks' ScalarE-bound softmax work.
  * Attention: 8 blocks of (head-pair, 512-query quarter).  Scores for the
    two heads of a pair target PE row groups 0/64 and run concurrently.
  * exp: ScalarE ACTIVATE for most key-tiles; a fixed subset is offloaded to
    VectorE as a Schraudolph bit-trick exp (affine fp32->int32 cast, then a
    bitcast copy to bf16), balancing the two engines.  The softmax
    normalization cancels most of the ~2% Schraudolph error.
  * Per-head-pair denominator batch: SBUF->SBUF DMA stages the two ones-row
    sums onto partition 0; one reciprocal_approx_fast inverts both.
  * Output projection + output DMA interleave per query quarter.
  * PSUM: proj/oproj share 2 banks, score tiles 2x2, av 2x1 -> exactly 8.
"""

import numpy as np

import concourse.bass as bass
import concourse.mybir as mybir
import concourse.tile as tile
from concourse import bacc
from concourse.bass_utils import run_bass_kernel_spmd

P = 128
S = 2048
D = 1024
KT = D // P          # 8 k-tiles over d_model
NH = 4               # heads per core
DK = 64
NCORES = 8
CH = 512             # column chunk (keys/queries) for DMA + proj + attention
NCH = S // CH        # 4 chunks
F32 = mybir.dt.float32
I32 = mybir.dt.int32
BF16 = mybir.dt.bfloat16
AF = mybir.ActivationFunctionType

# Schraudolph exp: exp(x) ~= bitcast_f32(int32(A*x + B)); applied on these
# key-tile indices (DVE), the rest stay on ScalarE's exact exp.
SCH_A = float(2 ** 23 / np.log(2.0))
SCH_B = float(127 * 2 ** 23 - 366000)
SCH_J = ()

_CACHE: dict = {}
LAST_RESULTS = None  # test harness peeks at this for exec_time_ns


def _build_nc():
    nc = bacc.Bacc("TRN2", target_bir_lowering=False, num_devices=NCORES)

    qT = nc.dram_tensor("qT", [D, S], BF16, kind="ExternalInput").ap()
    kT = nc.dram_tensor("kT", [D, S], BF16, kind="ExternalInput").ap()
    vT = nc.dram_tensor("vT", [D, S], BF16, kind="ExternalInput").ap()
    wq = nc.dram_tensor("wq", [D, NH * DK], BF16, kind="ExternalInput").ap()
    wk = nc.dram_tensor("wk", [D, NH * DK], BF16, kind="ExternalInput").ap()
    wv = nc.dram_tensor("wv", [D, NH * DK], BF16, kind="ExternalInput").ap()
    wo = nc.dram_tensor("wo", [NH * DK, D], BF16, kind="ExternalInput").ap()
    out = nc.dram_tensor("outT", [D, S], F32, kind="ExternalOutput").ap()

    with tile.TileContext(nc) as tc:
        _build_body(nc, tc, qT, kT, vT, wq, wk, wv, wo, out)
    nc.compile()
    return nc


def _build_body(nc, tc, qT, kT, vT, wq, wk, wv, wo, out):
    from contextlib import ExitStack

    with ExitStack() as ctx:
        constp = ctx.enter_context(tc.tile_pool(name="const", bufs=1))

        # ---- persistent SBUF tensors -----------------------------------
        wq_s = constp.tile([P, KT, NH * DK], BF16)
        wk_s = constp.tile([P, KT, NH * DK], BF16)
        wv_s = constp.tile([P, KT, NH * DK], BF16)
        wo_s = constp.tile([P, 2, D], BF16)

        qhT = constp.tile([P, 2, S], BF16)   # [2 heads stacked, pair, S]
        khT = constp.tile([P, 2, S], BF16)
        # vh + ones column per head: [s_tile_part, s_tile, head, dv+1]
        vh_s = constp.tile([P, 16, NH, DK + 1], BF16)
        nc.any.memset(vh_s[:, :, :, DK], 1.0)
        # pair-stacked scaled attention output, transposed: [2*dv, S]
        avT = [constp.tile([P, S], BF16, name=f"avT{pr}") for pr in range(2)]
        # softmax denominators, all staged to partition 0: [1, head, ihq, CH]
        den_all = constp.tile([1, NH, NCH, CH], F32)
        rec_all = constp.tile([1, NH, NCH, CH], F32)

        # ---- input DMA: first chunk of k/v/q first, q tails last -------
        xstream = ctx.enter_context(tc.tile_pool(name="xstream", bufs=6))
        xch: dict = {}

        def load_chunk(name, src_, c):
            cs = slice(c * CH, (c + 1) * CH)
            t = xstream.tile([P, KT, CH], BF16, tag="xs", name=f"x{name}{c}")
            nc.sync.dma_start(t, src_[:, cs].rearrange("(kt p) s -> p kt s", p=P))
            xch[name, c] = t

        nc.sync.dma_start(wk_s, wk.rearrange("(kt p) n -> p kt n", p=P))
        load_chunk("k", kT, 0)
        nc.sync.dma_start(wv_s, wv.rearrange("(kt p) n -> p kt n", p=P))
        load_chunk("v", vT, 0)
        nc.sync.dma_start(wq_s, wq.rearrange("(kt p) n -> p kt n", p=P))
        load_chunk("q", qT, 0)
        nc.sync.dma_start(wo_s, wo.rearrange("(pair p) n -> p pair n", p=P))
        for c in range(1, NCH):
            load_chunk("k", kT, c)
            load_chunk("v", vT, c)
        for c in range(1, NCH):
            load_chunk("q", qT, c)

        # pproj: shared 1-bank slots for q/k/v projections AND the output
        # projection (lifetimes interleave but tags share the 2 slots).
        pproj = ctx.enter_context(
            tc.tile_pool(name="pproj", bufs=2, space="PSUM"))
        pst = ctx.enter_context(tc.tile_pool(name="pst", bufs=2, space="PSUM"))
        pav = ctx.enter_context(tc.tile_pool(name="pav", bufs=1, space="PSUM"))
        attsb = ctx.enter_context(tc.tile_pool(name="attsb", bufs=5))
        nrmsb = ctx.enter_context(tc.tile_pool(name="nrmsb", bufs=2))
        schsb = ctx.enter_context(tc.tile_pool(name="schsb", bufs=2))
        osb = ctx.enter_context(tc.tile_pool(name="osb", bufs=2))

        def qk_proj(name, wsb, dst, c):
            xc = xch[name, c]
            cs = slice(c * CH, (c + 1) * CH)
            for pr in range(2):
                ps = pproj.tile([P, CH], F32, tag="pj", name="pj")
                for kt in range(KT):
                    nc.tensor.matmul(
                        ps,
                        wsb[:, kt, pr * P:(pr + 1) * P],
                        xc[:, kt, :],
                        start=(kt == 0),
                        stop=(kt == KT - 1),
                    )
                # all PSUM->SBUF copies stay off ScalarE: it runs exp only
                nc.vector.tensor_copy(dst[:, pr, cs], ps)

        def v_proj(c):
            # stationary = v chunk [128d, 128keys]; moving = Wv [128d, 256]
            xc = xch["v", c]
            for sst in range(CH // P):
                st = c * (CH // P) + sst
                vp = pproj.tile([P, NH * DK], F32, tag="pj", name="pj")
                for kt in range(KT):
                    nc.tensor.matmul(
                        vp,
                        xc[:, kt, sst * P:(sst + 1) * P],
                        wv_s[:, kt, :],
                        start=(kt == 0),
                        stop=(kt == KT - 1),
                    )
                src = vp.rearrange("p (h d) -> p h d", h=NH)
                nc.vector.tensor_copy(vh_s[:, st, :, 0:DK], src)

        # ---- attention block plumbing ----------------------------------
        LAG = 3

        def attn_start(pr, ihq):
            av_A = pav.tile([DK + 1, CH], F32, tag="avA", name="av_A")
            av_B = pav.tile([DK + 1, CH], F32, tag="avB", name="av_B")
            return {"pr": pr, "ihq": ihq, "av_A": av_A, "av_B": av_B,
                    "avq": []}

        def _av_mm(st, j, pt):
            pr = st["pr"]
            nc.tensor.matmul(
                st["av_A"],
                vh_s[:, j, 2 * pr, :], pt[:, 0:CH],
                start=(j == 0), stop=(j == 15),
            )
            nc.tensor.matmul(
                st["av_B"],
                vh_s[:, j, 2 * pr + 1, :], pt[:, CH:2 * CH],
                start=(j == 0), stop=(j == 15),
            )

        def attn_js(st, j0, j1):
            pr, ihq = st["pr"], st["ihq"]
            qs = slice(ihq * CH, (ihq + 1) * CH)
            for j in range(j0, j1):
                js = slice(j * P, (j + 1) * P)
                stq = pst.tile([P, 2 * CH], F32, tag="st", name="stq")
                # two heads -> different PE row groups, run concurrently
                nc.tensor.matmul(
                    stq[:, 0:CH],
                    khT[0:DK, pr, js], qhT[0:DK, pr, qs],
                    start=True, stop=True,
                )
                nc.tensor.matmul(
                    stq[:, CH:2 * CH],
                    khT[DK:P, pr, js], qhT[DK:P, pr, qs],
                    start=True, stop=True,
                )
                pt = attsb.tile([P, 2 * CH], BF16, tag="pt", name="pt")
                if j in SCH_J:
                    it = schsb.tile([P, 2 * CH], I32, tag="sch", name="it")
                    nc.vector.tensor_scalar(
                        out=it, in0=stq, scalar1=SCH_A, scalar2=SCH_B,
                        op0=mybir.AluOpType.mult, op1=mybir.AluOpType.add,
                    )
                    nc.vector.tensor_copy(pt, it.bitcast(F32))
                else:
                    nc.scalar.activation(pt, stq, AF.Exp)
                # av matmuls accumulate (commutative over j) and are emitted
                # LAG tiles late so a slow exp never head-of-line blocks the
                # PE FIFO (which would starve whichever engine exps next).
                st["avq"].append((j, pt))
                if len(st["avq"]) > LAG:
                    _av_mm(st, *st["avq"].pop(0))

        def attn_finish(st):
            pr, ihq = st["pr"], st["ihq"]
            qs = slice(ihq * CH, (ihq + 1) * CH)
            for item in st["avq"]:
                _av_mm(st, *item)
            for h, av in enumerate((st["av_A"], st["av_B"])):
                r = 2 * pr + h
                av_sb = nrmsb.tile([DK + 1, CH], F32, tag=f"avsb{r}",
                                   name="av_sb")
                nc.vector.tensor_copy(av_sb, av)
                # stage the denominator row (partition 64 -> partition 0)
                nc.sync.dma_start(den_all[0:1, r, ihq, :],
                                  av_sb[DK:DK + 1, :])
                xch[("avsb", pr, h)] = av_sb
            # per-pair reciprocal so pr0's normalization overlaps pr1's
            # attention instead of waiting for the whole quarter
            nc.vector.reciprocal_approx_fast(
                rec_all[0:1, 2 * pr:2 * pr + 2, ihq, :],
                den_all[0:1, 2 * pr:2 * pr + 2, ihq, :])
            for h in range(2):
                r = 2 * pr + h
                av_sb = xch[("avsb", pr, h)]
                bcs = nrmsb.tile([DK, CH], F32, tag="bcs", name="bcs")
                nc.gpsimd.partition_broadcast(bcs, rec_all[0:1, r, ihq, :])
                nc.vector.tensor_mul(
                    out=avT[pr][h * DK:(h + 1) * DK, qs],
                    in0=av_sb[0:DK, :],
                    in1=bcs,
                )

        def oproj(ihq):
            qs = slice(ihq * CH, (ihq + 1) * CH)
            for dc in range(8):
                ds_ = slice(dc * P, (dc + 1) * P)
                ops = pproj.tile([P, CH], F32, tag="pj", name="pj")
                for pair in range(2):
                    nc.tensor.matmul(
                        ops,
                        wo_s[:, pair, ds_],
                        avT[pair][:, qs],
                        start=(pair == 0), stop=(pair == 1),
                    )
                oto = osb.tile([P, CH], F32, tag="oto", name="oto")
                nc.vector.tensor_copy(oto, ops)
                nc.sync.dma_start(out[ds_, qs], oto)

        # ---- fused schedule --------------------------------------------
        # quarter 0 / pair 0 chases the k/v projection chunk by chunk: its
        # first score tiles only need keys 0:512, so ScalarE starts exping
        # ~30us before the projections finish.
        kv_done = [False] * NCH

        def kv_chunk(c):
            if not kv_done[c]:
                qk_proj("k", wk_s, khT, c)
                v_proj(c)
                kv_done[c] = True

        kv_chunk(0)
        qk_proj("q", wq_s, qhT, 0)
        st00 = attn_start(0, 0)
        for c in range(1, NCH):
            attn_js(st00, (c - 1) * 4, c * 4)
            kv_chunk(c)
        attn_js(st00, 12, 16)
        attn_finish(st00)

        for ihq in range(NCH):          # query quarter (512 queries)
            if ihq > 0:
                qk_proj("q", wq_s, qhT, ihq)
                st = attn_start(0, ihq)
                attn_js(st, 0, 16)
                attn_finish(st)
            if ihq > 0:
                oproj(ihq - 1)
            st = attn_start(1, ihq)
            attn_js(st, 0, 16)
            attn_finish(st)
        oproj(NCH - 1)


def kernel(q, k, v, mask, Wq, Wk, Wv, Wo, _trace=False, _tmpdir=None):
    """Full inputs in, full output out. mask is all-True by construction of
    the problem's input spec and is ignored (dense softmax)."""
    global LAST_RESULTS

    import ml_dtypes

    bf16 = ml_dtypes.bfloat16
    q = np.asarray(q, dtype=np.float32)
    k = np.asarray(k, dtype=np.float32)
    v = np.asarray(v, dtype=np.float32)
    Wq = np.asarray(Wq, dtype=bf16)
    Wk = np.asarray(Wk, dtype=bf16)
    Wv = np.asarray(Wv, dtype=bf16)
    Wo = np.asarray(Wo, dtype=bf16)
    B = q.shape[0]

    if "nc" not in _CACHE:
        _CACHE["nc"] = _build_nc()
    nc = _CACHE["nc"]

    qTb = [np.ascontiguousarray(q[b].T).astype(bf16) for b in range(B)]
    kTb = [np.ascontiguousarray(k[b].T).astype(bf16) for b in range(B)]
    vTb = [np.ascontiguousarray(v[b].T).astype(bf16) for b in range(B)]

    in_maps = []
    for core in range(NCORES):
        b, hg = core // 4, core % 4
        cs = slice(hg * NH * DK, (hg + 1) * NH * DK)
        in_maps.append({
            "qT": qTb[b],
            "kT": kTb[b],
            "vT": vTb[b],
            "wq": np.ascontiguousarray(Wq[:, cs]),
            "wk": np.ascontiguousarray(Wk[:, cs]),
            "wv": np.ascontiguousarray(Wv[:, cs]),
            "wo": np.ascontiguousarray(Wo[cs, :]),
        })

    res = run_bass_kernel_spmd(
        nc, in_maps, core_ids=list(range(NCORES)),
        trace=_trace, tmpdir=_tmpdir,
    )
    LAST_RESULTS = res

    fullT = np.zeros((B, D, S), dtype=np.float32)
    for core in range(NCORES):
        fullT[core // 4] += res.results[core]["outT"]
    return np.ascontiguousarray(fullT.transpose(0, 2, 1))
